# revision 7
# baseline (speedup 1.0000x reference)
"""Trainium2 Bass kernel for nn_CRModule (retrieval_knn).

reference:
    xf = x.reshape(4096, 4096); xa = xf[:, ::2]; xb = xf[:, 1::2]   # [T=4096, 2048]
    sq[i,j] = |xa[:,i]|^2 + |xb[:,j]|^2 - 2 * xa[:,i].xb[:,j]
    wsum = fc_weight.sum(0); wa = wsum[::2]; wb = wsum[1::2]
    scores[i,j] = ((wa[i] + wb[j]) * sqrt(max(sq,0)))**2
                = (wa[i] + wb[j])**2 * max(sq[i,j], 0)     # sqrt cancels

Strategy (single SPMD launch, 8 cores in a 4x2 grid, d = 4*c + r):
  core owns scores rows [512r, 512r+512) x cols [1024c, 1024c+1024).
  - main matmul (-2a)^T b in fp8 DoubleRow mode (2 k-tiles/instr).
  - fc column sums OFF the PE: host supplies fc^T [128, 4, 12288] bf16,
    DVE tensor_reduce along the free axis -> [128, 4] partials
    (tiles 0..1 = wa slice for own rows, 2..3 = wb shard).
  - nb from a transposed xb shard (scalar square + DVE reduce).
  - na on PE from bf16 squares of the (-2a) tiles (cheap, early).
  - collectives: dummy AllGather at t=0 (absorbs CC stream init),
    wa pair-exchange groups [[0,4],[1,5],[2,6],[3,7]],
    fused [wb|nb] gather groups [[0,1,2,3],[4,5,6,7]].
  - epilogue in PSUM: (ps + na + nb) -> relu -> * (wa+wb)^2 -> bf16 out.
"""

import numpy as np
import ml_dtypes

import concourse.bass as bass
import concourse.tile as tile
from concourse import bacc, mybir
from concourse.bass_utils import run_bass_kernel_spmd

BF16 = mybir.dt.bfloat16
F32 = mybir.dt.float32
FP8 = mybir.dt.float8e4
NP_BF16 = ml_dtypes.bfloat16
NP_FP8 = ml_dtypes.float8_e4m3
DR = mybir.MatmulPerfMode.DoubleRow

D = 8          # cores
GR, GC = 4, 2  # grid: d = 4*c + r
T = 4096       # contraction dim = B*N
KT = T // 128  # 32 k-tiles
KP = KT // 2   # 16 DoubleRow k-pairs
CA = 2048      # C/2 channels
MR = CA // GR  # 512 output rows per core  (4 m-tiles)
NCL = CA // GC  # 1024 output cols per core
MT = MR // 128  # 4 m-tiles
NJ = NCL // 512  # 2 psum column chunks
O = 12288      # fc rows
C = 4096

XCH = 8        # xa chunks   [128, 4, MR]   fp8   (4 k-tiles each)
BCH = 8        # xbr chunks  [128, 4, NCL]  fp8
FCH = 12       # fcT chunks  [128, 4, O//FCH] bf16
FCW = O // FCH  # 1024 fc rows per chunk

_cache = {}


def _new_nc():
    return bacc.Bacc("TRN2", target_bir_lowering=False, debug=False, num_devices=D)


def _build_v2():
    nc = _new_nc()
    xasc_d = nc.dram_tensor("xasc", [128, KT, MR], FP8, kind="ExternalInput").ap()
    xbr_d = nc.dram_tensor("xbr", [128, KT, NCL], FP8, kind="ExternalInput").ap()
    xbsT_d = nc.dram_tensor("xbsT", [128, 2, T], FP8, kind="ExternalInput").ap()
    fcT_d = nc.dram_tensor("fcT", [128, 4, O], BF16, kind="ExternalInput").ap()
    out_d = nc.dram_tensor("scores", [MR, NCL], BF16, kind="ExternalOutput").ap()

    dum_in = nc.dram_tensor("dum_in", [1, 8], F32).ap()
    dum_sh = nc.dram_tensor("dum_sh", [D, 8], F32, addr_space="Shared").ap()
    wa_in = nc.dram_tensor("wa_in", [1, 256], F32).ap()
    wa_sh = nc.dram_tensor("wa_sh", [GC, 256], F32).ap()
    wbnb_in = nc.dram_tensor("wbnb_in", [1, 512], F32).ap()
    wbnb_sh = nc.dram_tensor("wbnb_sh", [GR, 512], F32).ap()

    grp_all = [list(range(D))]
    grp_wa = [[r, r + 4] for r in range(4)]        # same r, c = 0|1
    grp_wbnb = [[0, 1, 2, 3], [4, 5, 6, 7]]        # same c, pos = r

    import contextlib
    with tile.TileContext(nc) as tc:
        es = contextlib.ExitStack()
        with es, \
             tc.tile_pool(name="xap", bufs=1) as xap, \
             tc.tile_pool(name="xbp", bufs=1) as xbp, \
             tc.tile_pool(name="xtp", bufs=1) as xtp, \
             tc.tile_pool(name="fcp", bufs=4) as fcp, \
             tc.tile_pool(name="small", bufs=1) as small, \
             tc.tile_pool(name="x2p", bufs=2) as x2p, \
             tc.tile_pool(name="sqp", bufs=1) as sqp, \
             tc.tile_pool(name="w2p", bufs=1) as w2p, \
             tc.tile_pool(name="outp", bufs=2) as outp:
            psna = es.enter_context(tc.tile_pool(name="psna", bufs=1, space="PSUM"))
            psnt = es.enter_context(tc.tile_pool(name="psnt", bufs=1, space="PSUM"))

            # ---- dummy collective: absorb CC stream init barrier at t=0 ----
            dumt = small.tile([1, 8], F32)
            nc.vector.memset(dumt[:], 0.0)
            nc.gpsimd.dma_start(dum_in[:], dumt[:])
            nc.gpsimd.collective_compute(
                "AllGather", mybir.AluOpType.bypass, replica_groups=grp_all,
                ins=[dum_in[:]], outs=[dum_sh[:]])

            # ---- DMA emission order == queue service order ----
            xac = []
            for i in range(XCH):
                x_t = xap.tile([128, 4, MR], FP8, tag=f"xa{i}")
                nc.sync.dma_start(x_t[:], xasc_d[:, 4 * i:4 * i + 4, :])
                xac.append(x_t)
            xbsT_sb = xtp.tile([128, 2, T], FP8)
            nc.sync.dma_start(xbsT_sb[:], xbsT_d[:])
            # interleave xbr (mm stream) with fcT (DVE reduce stream);
            # fcT finishes first so the AllGather tail hides under mm.
            ft = []
            xbt = []
            order = ["x0", "f0", "x1", "f1", "x2", "f2", "x3", "f3",
                     "x4", "f4", "x5", "f5", "f6", "x6", "f7", "f8",
                     "f9", "f10", "f11", "x7"]
            for item in order:
                idx = int(item[1:])
                if item[0] == "x":
                    xb_t = xbp.tile([128, 4, NCL], FP8, tag=f"xb{idx}")
                    nc.sync.dma_start(xb_t[:], xbr_d[:, 4 * idx:4 * idx + 4, :])
                    xbt.append((idx, xb_t))
                else:
                    f_t = fcp.tile([128, 4, FCW], BF16, tag="fc")
                    nc.sync.dma_start(
                        f_t[:], fcT_d[:, :, FCW * idx:FCW * (idx + 1)])
                    ft.append(f_t)
            xbt = [t for _, t in sorted(xbt)]

            ones2 = small.tile([128, 1], BF16)
            nc.vector.memset(ones2[:], 0.25)
            onef = small.tile([1, 1], F32)
            nc.vector.memset(onef[:], 1.0)

            # ---- na: squares (scalar, bf16) + PE column sums ----
            psa = psna.tile([1, MR], F32)
            for i in range(XCH):
                x2t = x2p.tile([128, 4, MR], BF16, tag="x2")
                nc.scalar.square(x2t[:], xac[i][:])
                for s in range(4):
                    kt = 4 * i + s
                    nc.tensor.matmul(
                        psa[:], ones2[:], x2t[:, s, :],
                        start=(kt == 0), stop=(kt == KT - 1))
            nast = small.tile([1, MR], F32)
            nc.vector.tensor_copy(nast[:], psa[:])
            # transpose na [1, MR] -> [128, MT] via K=1 matmuls
            pst = psnt.tile([128, MT], F32)
            for m in range(MT):
                nc.tensor.matmul(
                    pst[:, m:m + 1], nast[0:1, m * 128:(m + 1) * 128], onef[:],
                    start=(m == 0), stop=(m == MT - 1), skip_group_check=True)
            nav = small.tile([128, MT], F32)
            nc.vector.tensor_copy(nav[:], pst[:])

            # ---- nb: scalar square + DVE reduce over free axis ----
            xbsq = sqp.tile([128, 2, T], BF16)
            nc.scalar.square(xbsq[:], xbsT_sb[:])
            nbsum = small.tile([128, 2, 1], F32)
            nc.vector.tensor_reduce(
                nbsum[:], xbsq[:], mybir.AxisListType.X,
                mybir.AluOpType.add)

            # ---- fc column sums on DVE (chunk reduces + final fold) ----
            fred = small.tile([128, 4, FCH], F32)
            for ch in range(FCH):
                nc.vector.tensor_reduce(
                    fred[:, :, ch:ch + 1], ft[ch][:], mybir.AxisListType.X,
                    mybir.AluOpType.add)
            fsum = small.tile([128, 4, 1], F32)
            nc.vector.tensor_reduce(
                fsum[:], fred[:], mybir.AxisListType.X,
                mybir.AluOpType.add)

            # ---- collectives (gpsimd): wa pair-exchange + fused [wb|nb] ----
            nc.gpsimd.dma_start(
                bass.AP(tensor=wa_in.tensor, offset=0, ap=[[1, 128], [128, 2]]),
                fsum[:, 0:2, 0])
            nc.gpsimd.collective_compute(
                "AllGather", mybir.AluOpType.bypass, replica_groups=grp_wa,
                ins=[wa_in[:]], outs=[wa_sh[:]])
            nc.gpsimd.dma_start(
                bass.AP(tensor=wbnb_in.tensor, offset=0, ap=[[1, 128], [128, 2]]),
                fsum[:, 2:4, 0])
            nc.gpsimd.dma_start(
                bass.AP(tensor=wbnb_in.tensor, offset=256, ap=[[1, 128], [128, 2]]),
                nbsum[:, :, 0])
            nc.gpsimd.collective_compute(
                "AllGather", mybir.AluOpType.bypass, replica_groups=grp_wbnb,
                ins=[wbnb_in[:]], outs=[wbnb_sh[:]])
            wav = small.tile([128, MT], F32)
            nc.gpsimd.dma_start(
                wav[:],
                bass.AP(tensor=wa_sh.tensor, offset=0, ap=[[1, 128], [128, MT]]))
            wb_bc = small.tile([128, NCL], F32)
            nc.gpsimd.dma_start(
                wb_bc[:],
                bass.AP(tensor=wbnb_sh.tensor, offset=0,
                        ap=[[0, 128], [512, GR], [1, 256]]))
            nb_bc = small.tile([128, NCL], F32)
            nc.gpsimd.dma_start(
                nb_bc[:],
                bass.AP(tensor=wbnb_sh.tensor, offset=256,
                        ap=[[0, 128], [512, GR], [1, 256]]))

            # ---- w2[m] = (wa_m + wb)^2 on scalar engine ----
            w2 = []
            for m in range(MT):
                w2m = w2p.tile([128, NCL], F32, tag=f"w2_{m}")
                nc.scalar.activation(
                    w2m[:], wb_bc[:], mybir.ActivationFunctionType.Square,
                    bias=wav[:, m:m + 1], scale=1.0)
                w2.append(w2m)

            # ---- main matmul: fp8 DoubleRow, k-outer, 8 psum banks ----
            es.close()
            with tc.tile_pool(name="psmm", bufs=1, space="PSUM") as psmm:
                ps = [psmm.tile([128, NJ, 512], F32, name=f"ps{m}", tag="ps")
                      for m in range(MT)]
                for kp in range(KP):
                    ch, s = divmod(kp, 2)
                    for m in range(MT):
                        for nj in range(NJ):
                            nc.tensor.matmul(
                                ps[m][:, nj, :],
                                xac[ch][:, 2 * s:2 * s + 2, m * 128:(m + 1) * 128],
                                xbt[ch][:, 2 * s:2 * s + 2, nj * 512:(nj + 1) * 512],
                                perf_mode=DR,
                                start=(kp == 0), stop=(kp == KP - 1))

                # ---- epilogue: (ps + na + nb) relu * w2 -> bf16 out ----
                for m in range(MT):
                    pflat = ps[m].rearrange("p a b -> p (a b)")
                    nc.vector.scalar_tensor_tensor(
                        pflat, pflat, nav[:, m:m + 1], nb_bc[:],
                        op0=mybir.AluOpType.add, op1=mybir.AluOpType.add)
                    ot = outp.tile([128, NCL], BF16, tag="ot")
                    nc.vector.scalar_tensor_tensor(
                        ot[:], pflat, 0.0, w2[m][:],
                        op0=mybir.AluOpType.max, op1=mybir.AluOpType.mult)
                    nc.sync.dma_start(out_d[m * 128:(m + 1) * 128, :], ot[:])

    nc.compile()
    return nc


def _p_major(a, np_dtype):
    """[n*128, cols] -> [128, n, cols] with tile index in the middle."""
    n = a.shape[0] // 128
    return np.ascontiguousarray(
        a.reshape(n, 128, a.shape[1]).transpose(1, 0, 2).astype(np_dtype)
    )


def _run_v2(x, fc_weight, _trace=False):
    x = np.asarray(x, dtype=np.float32)
    fc = np.asarray(fc_weight, dtype=np.float32)
    xf = x.reshape(T, C)
    xa = np.ascontiguousarray(xf[:, 0::2])   # [T, CA]
    xb = np.ascontiguousarray(xf[:, 1::2])
    xa_s2 = -2.0 * xa

    if "v2" not in _cache:
        _cache["v2"] = _build_v2()
    ncm = _cache["v2"]

    in_maps = []
    for d in range(D):
        c, r = divmod(d, 4)
        rs = slice(512 * r, 512 * r + 512)
        cs = slice(1024 * c, 1024 * c + 1024)
        nbs = slice(1024 * c + 256 * r, 1024 * c + 256 * r + 256)
        # fcT: even cols for own rows [512r+256c, +256), odd cols for
        # wb shard [1024c+256r, +256); stacked, transposed, p-major.
        ev0 = 2 * (512 * r + 256 * c)
        od0 = 2 * (1024 * c + 256 * r) + 1
        fc_ev = fc[:, ev0:ev0 + 512:2].T          # [256, O]
        fc_od = fc[:, od0:od0 + 512:2].T          # [256, O]
        fcT = np.concatenate([fc_ev, fc_od], axis=0)  # [512, O]
        in_maps.append({
            "xasc": _p_major(xa_s2[:, rs], NP_FP8),
            "xbr": _p_major(xb[:, cs], NP_FP8),
            "xbsT": _p_major(np.ascontiguousarray(xb[:, nbs].T), NP_FP8),
            "fcT": _p_major(np.ascontiguousarray(fcT), NP_BF16),
        })

    res = run_bass_kernel_spmd(ncm, in_maps, core_ids=list(range(D)), trace=_trace)
    out = np.zeros((CA, CA), dtype=np.float32)
    for d in range(D):
        c, r = divmod(d, 4)
        out[512 * r:512 * r + 512, 1024 * c:1024 * c + 1024] = \
            res.results[d]["scores"].astype(np.float32)
    if _trace:
        _run_v2.last_times = (res.exec_time_ns,)
    return out


def kernel(x, fc_weight):
    """Graded entrypoint: full inputs in, full [2048, 2048] scores out."""
    return _run_v2(x, fc_weight)


# revision 16
# speedup vs baseline: 1.1925x; 1.1925x over previous
"""Trainium2 Bass kernel for nn_CRModule (retrieval_knn).

reference:
    xf = x.reshape(4096, 4096); xa = xf[:, ::2]; xb = xf[:, 1::2]   # [T=4096, 2048]
    sq[i,j] = |xa[:,i]|^2 + |xb[:,j]|^2 - 2 * xa[:,i].xb[:,j]
    wsum = fc_weight.sum(0); wa = wsum[::2]; wb = wsum[1::2]
    scores[i,j] = (wa[i] + wb[j])**2 * max(sq[i,j], 0)

Strategy (single SPMD launch, 8 cores in a 4x2 grid, d = 4*c + r):
  core owns scores rows [512r, 512r+512) x cols [1024c, 1024c+1024).
  - main matmul (-2a)^T b in fp8 DoubleRow mode (2 k-tiles/instr).
  - fc column sums OFF the PE: host supplies fc^T bf16 (wb half first,
    wa half second), DVE tensor_reduce (bf16 chunk outs for 2x mode).
  - nb via scalar activation(Square, accum_out) on a transposed xb shard.
  - na on PE from bf16 squares; na+nb injected into PSUM via K=2 matmuls
    (row0=na/ones, row1=ones/nb), so the epilogue is one stt per m-tile.
  - collectives: [wb|nb] gather groups [[0,1,2,3],[4,5,6,7]] fired early,
    wa pair-exchange groups [[0,4],[1,5],[2,6],[3,7]] second.
  - epilogue: relu(psum) * (wa+wb)^2 -> bf16 out.
"""

import numpy as np
import ml_dtypes

import concourse.bass as bass
import concourse.tile as tile
from concourse import bacc, mybir
from concourse.bass_utils import run_bass_kernel_spmd

BF16 = mybir.dt.bfloat16
F32 = mybir.dt.float32
FP8 = mybir.dt.float8e4
NP_BF16 = ml_dtypes.bfloat16
NP_FP8 = ml_dtypes.float8_e4m3
DR = mybir.MatmulPerfMode.DoubleRow

D = 8          # cores
T = 4096       # contraction dim = B*N
KT = T // 128  # 32 k-tiles
KP = KT // 2   # 16 DoubleRow k-pairs
CA = 2048      # C/2 channels
MR = 512       # output rows per core  (4 m-tiles)
NCL = 1024     # output cols per core
MT = MR // 128  # 4 m-tiles
NJ = NCL // 512  # 2 psum column chunks
O = 12288      # fc rows
C = 4096

XCH = 8        # xa chunks   [128, 4, MR]  fp8
BCH = 8        # xbr chunks  [128, 4, NCL] fp8
FHH = 6        # fcT chunks per half [128, 2, FCW] bf16
FCW = O // FHH  # 2048 fc rows per chunk

_cache = {}


def _new_nc():
    return bacc.Bacc("TRN2", target_bir_lowering=False, debug=False, num_devices=D)


def _build_v3():
    nc = _new_nc()
    xasc_d = nc.dram_tensor("xasc", [128, KT, MR], FP8, kind="ExternalInput").ap()
    xbr_d = nc.dram_tensor("xbr", [128, KT, NCL], FP8, kind="ExternalInput").ap()
    xbsT_d = nc.dram_tensor("xbsT", [128, 2, T], FP8, kind="ExternalInput").ap()
    # fc^T halves: b = odd cols (wb shard), a = even cols (own rows' wa)
    fcb_d = nc.dram_tensor("fcbT", [128, 2, O], BF16, kind="ExternalInput").ap()
    fca_d = nc.dram_tensor("fcaT", [128, 2, O], BF16, kind="ExternalInput").ap()
    out_d = nc.dram_tensor("scores", [MR, NCL], BF16, kind="ExternalOutput").ap()

    wa_in = nc.dram_tensor("wa_in", [1, 256], F32).ap()
    wa_sh = nc.dram_tensor("wa_sh", [2, 256], F32).ap()
    wbnb_in = nc.dram_tensor("wbnb_in", [1, 512], F32).ap()
    wbnb_sh = nc.dram_tensor("wbnb_sh", [4, 512], F32).ap()

    grp_wa = [[r, r + 4] for r in range(4)]        # same r, c = 0|1
    grp_wbnb = [[0, 1, 2, 3], [4, 5, 6, 7]]        # same c, pos = r

    import contextlib
    with tile.TileContext(nc) as tc:
        es = contextlib.ExitStack()
        with es, \
             tc.tile_pool(name="xap", bufs=1) as xap, \
             tc.tile_pool(name="xbp", bufs=1) as xbp, \
             tc.tile_pool(name="xtp", bufs=1) as xtp, \
             tc.tile_pool(name="fcp", bufs=4) as fcp, \
             tc.tile_pool(name="small", bufs=1) as small, \
             tc.tile_pool(name="x2p", bufs=2) as x2p, \
             tc.tile_pool(name="w2p", bufs=1) as w2p, \
             tc.tile_pool(name="outp", bufs=2) as outp:
            psna = es.enter_context(tc.tile_pool(name="psna", bufs=1, space="PSUM"))

            # ---- DMA emission order == queue service order ----
            xac = []
            for i in range(XCH):
                x_t = xap.tile([128, 4, MR], FP8, tag=f"xa{i}")
                nc.sync.dma_start(x_t[:], xasc_d[:, 4 * i:4 * i + 4, :])
                xac.append(x_t)
            xbsT_sb = xtp.tile([128, 2, T], FP8)
            nc.sync.dma_start(xbsT_sb[:], xbsT_d[:])
            fbt = []
            for i in range(FHH):
                f_t = fcp.tile([128, 2, FCW], BF16, tag="fc")
                nc.sync.dma_start(f_t[:], fcb_d[:, :, FCW * i:FCW * (i + 1)])
                fbt.append(f_t)
            fat = []
            for i in range(FHH):
                f_t = fcp.tile([128, 2, FCW], BF16, tag="fc")
                nc.sync.dma_start(f_t[:], fca_d[:, :, FCW * i:FCW * (i + 1)])
                fat.append(f_t)
            xbt = []
            for i in range(BCH):
                xb_t = xbp.tile([128, 4, NCL], FP8, tag=f"xb{i}")
                nc.sync.dma_start(xb_t[:], xbr_d[:, 4 * i:4 * i + 4, :])
                xbt.append(xb_t)

            ones2 = small.tile([128, 1], BF16)
            nc.vector.memset(ones2[:], 0.25)

            # ---- na: squares (scalar, bf16) + PE column sums ----
            psa = psna.tile([1, MR], F32)
            for i in range(XCH):
                x2t = x2p.tile([128, 4, MR], BF16, tag="x2")
                nc.scalar.square(x2t[:], xac[i][:])
                for s in range(4):
                    kt = 4 * i + s
                    nc.tensor.matmul(
                        psa[:], ones2[:], x2t[:, s, :],
                        start=(kt == 0), stop=(kt == KT - 1))
            # K=2 injection tile: row0 = na (bf16), row1 = ones
            ones_row = small.tile([1, NCL], BF16)
            nc.vector.memset(ones_row[:], 1.0)
            nak2 = small.tile([2, MR], BF16)
            nc.vector.tensor_copy(nak2[0:1, :], psa[:])
            nc.sync.dma_start(nak2[1:2, :], ones_row[0:1, 0:MR])

            # ---- nb: scalar square+accum (no DVE, no xbsq tile) ----
            nbP = small.tile([128, 2], F32)
            sqtrash = x2p.tile([128, T], BF16, tag="sqt")
            for t in range(2):
                nc.scalar.activation(
                    sqtrash[:], xbsT_sb[:, t, :],
                    mybir.ActivationFunctionType.Square,
                    accum_out=nbP[:, t:t + 1])

            # ---- fc sums on DVE: bf16 chunk reduces (2x mode attempt) ----
            with nc.allow_low_precision(reason="bf16 fc chunk partials, ~0.4% rel"):
                fredb = small.tile([128, FHH, 2], BF16)
                for chn in range(FHH):
                    nc.vector.tensor_reduce(
                        fredb[:, chn, :], fbt[chn][:], mybir.AxisListType.X,
                        mybir.AluOpType.add)
                freda = small.tile([128, FHH, 2], BF16)
                for chn in range(FHH):
                    nc.vector.tensor_reduce(
                        freda[:, chn, :], fat[chn][:], mybir.AxisListType.X,
                        mybir.AluOpType.add)
            # fold chunk partials (f32 outs); view [128, FHH, 2] as [128, 2, FHH]
            fbv = fredb[:]
            fsb = small.tile([128, 2, 1], F32)
            nc.vector.tensor_reduce(
                fsb[:], bass.AP(tensor=fbv.tensor, offset=fbv.offset,
                                ap=[list(fbv.ap[0]), [1, 2], [2, FHH]]),
                mybir.AxisListType.X, mybir.AluOpType.add)
            fav = freda[:]
            fsa = small.tile([128, 2, 1], F32)
            nc.vector.tensor_reduce(
                fsa[:], bass.AP(tensor=fav.tensor, offset=fav.offset,
                                ap=[list(fav.ap[0]), [1, 2], [2, FHH]]),
                mybir.AxisListType.X, mybir.AluOpType.add)

            # ---- collectives: [wb|nb] first (early), wa second ----
            nc.gpsimd.dma_start(
                bass.AP(tensor=wbnb_in.tensor, offset=0, ap=[[1, 128], [128, 2]]),
                fsb[:, :, 0])
            nc.gpsimd.dma_start(
                bass.AP(tensor=wbnb_in.tensor, offset=256, ap=[[1, 128], [128, 2]]),
                nbP[:])
            nc.gpsimd.collective_compute(
                "AllGather", mybir.AluOpType.bypass, replica_groups=grp_wbnb,
                ins=[wbnb_in[:]], outs=[wbnb_sh[:]])
            nc.gpsimd.dma_start(
                bass.AP(tensor=wa_in.tensor, offset=0, ap=[[1, 128], [128, 2]]),
                fsa[:, :, 0])
            nc.gpsimd.collective_compute(
                "AllGather", mybir.AluOpType.bypass, replica_groups=grp_wa,
                ins=[wa_in[:]], outs=[wa_sh[:]])
            # K=2 injection tile: row0 = ones, row1 = nb (bf16)
            nbrow_f = small.tile([1, NCL], F32)
            nc.gpsimd.dma_start(
                nbrow_f[:],
                bass.AP(tensor=wbnb_sh.tensor, offset=256,
                        ap=[[0, 1], [512, 4], [1, 256]]))
            nbrow_b = small.tile([1, NCL], BF16)
            nc.vector.tensor_copy(nbrow_b[:], nbrow_f[:])
            nbk2 = small.tile([2, NCL], BF16)
            nc.sync.dma_start(nbk2[0:1, :], ones_row[0:1, 0:NCL])
            nc.sync.dma_start(nbk2[1:2, :], nbrow_b[0:1, :])
            wb_bc = small.tile([128, NCL], F32)
            nc.gpsimd.dma_start(
                wb_bc[:],
                bass.AP(tensor=wbnb_sh.tensor, offset=0,
                        ap=[[0, 128], [512, 4], [1, 256]]))
            wav = small.tile([128, MT], F32)
            nc.gpsimd.dma_start(
                wav[:],
                bass.AP(tensor=wa_sh.tensor, offset=0, ap=[[1, 128], [128, MT]]))

            # ---- w2[m] = (wa_m + wb)^2 on scalar engine ----
            w2 = []
            for m in range(MT):
                w2m = w2p.tile([128, NCL], F32, tag=f"w2_{m}")
                nc.scalar.activation(
                    w2m[:], wb_bc[:], mybir.ActivationFunctionType.Square,
                    bias=wav[:, m:m + 1], scale=1.0)
                w2.append(w2m)

            # ---- main matmul: fp8 DoubleRow + K=2 na/nb injection ----
            es.close()
            with tc.tile_pool(name="psmm", bufs=4, space="PSUM") as psmm:
                ps = [psmm.tile([128, NJ, 512], F32, name=f"ps{m}", tag="ps")
                      for m in range(MT)]
                for kp in range(KP):
                    chn, s = divmod(kp, 2)
                    for m in range(MT):
                        for nj in range(NJ):
                            nc.tensor.matmul(
                                ps[m][:, nj, :],
                                xac[chn][:, 2 * s:2 * s + 2, m * 128:(m + 1) * 128],
                                xbt[chn][:, 2 * s:2 * s + 2, nj * 512:(nj + 1) * 512],
                                perf_mode=DR,
                                start=(kp == 0), stop=False)
                for m in range(MT):
                    for nj in range(NJ):
                        nc.tensor.matmul(
                            ps[m][:, nj, :],
                            nak2[:, m * 128:(m + 1) * 128],
                            nbk2[:, nj * 512:(nj + 1) * 512],
                            start=False, stop=True)

                # ---- epilogue: relu(ps) * w2 -> bf16 out ----
                for m in range(MT):
                    pflat = ps[m].rearrange("p a b -> p (a b)")
                    ot = outp.tile([128, NCL], BF16, tag="ot")
                    nc.vector.scalar_tensor_tensor(
                        ot[:], pflat, 0.0, w2[m][:],
                        op0=mybir.AluOpType.max, op1=mybir.AluOpType.mult)
                    nc.sync.dma_start(out_d[m * 128:(m + 1) * 128, :], ot[:])

    nc.compile()
    return nc


def _p_major(a, np_dtype):
    """[n*128, cols] -> [128, n, cols] with tile index in the middle."""
    n = a.shape[0] // 128
    return np.ascontiguousarray(
        a.reshape(n, 128, a.shape[1]).transpose(1, 0, 2).astype(np_dtype)
    )


def _make_in_maps(x, fc_weight):
    x = np.asarray(x, dtype=np.float32)
    fc = np.asarray(fc_weight, dtype=np.float32)
    xf = x.reshape(T, C)
    xa = np.ascontiguousarray(xf[:, 0::2])   # [T, CA]
    xb = np.ascontiguousarray(xf[:, 1::2])
    xa_s2 = -2.0 * xa

    in_maps = []
    for d in range(D):
        c, r = divmod(d, 4)
        rs = slice(512 * r, 512 * r + 512)
        cs = slice(1024 * c, 1024 * c + 1024)
        nbs = slice(1024 * c + 256 * r, 1024 * c + 256 * r + 256)
        ev0 = 2 * (512 * r + 256 * c)            # even cols for own rows
        od0 = 2 * (1024 * c + 256 * r) + 1       # odd cols (wb shard)
        fc_ev = np.ascontiguousarray(fc[:, ev0:ev0 + 512:2].T)  # [256, O]
        fc_od = np.ascontiguousarray(fc[:, od0:od0 + 512:2].T)  # [256, O]
        in_maps.append({
            "xasc": _p_major(xa_s2[:, rs], NP_FP8),
            "xbr": _p_major(xb[:, cs], NP_FP8),
            "xbsT": _p_major(np.ascontiguousarray(xb[:, nbs].T), NP_FP8),
            "fcbT": _p_major(fc_od, NP_BF16),
            "fcaT": _p_major(fc_ev, NP_BF16),
        })
    return in_maps


def _run_v3(x, fc_weight, _trace=False):
    if "v3" not in _cache:
        _cache["v3"] = _build_v3()
    ncm = _cache["v3"]
    in_maps = _make_in_maps(x, fc_weight)

    res = run_bass_kernel_spmd(ncm, in_maps, core_ids=list(range(D)), trace=_trace)
    out = np.zeros((CA, CA), dtype=np.float32)
    for d in range(D):
        c, r = divmod(d, 4)
        out[512 * r:512 * r + 512, 1024 * c:1024 * c + 1024] = \
            res.results[d]["scores"].astype(np.float32)
    if _trace:
        _run_v3.last_times = (res.exec_time_ns,)
    return out


_run_v2 = _run_v3  # test.py compatibility


def kernel(x, fc_weight):
    """Graded entrypoint: full inputs in, full [2048, 2048] scores out."""
    return _run_v3(x, fc_weight)


# revision 19
# speedup vs baseline: 1.4824x; 1.2431x over previous
"""Trainium2 Bass kernel for nn_CRModule (retrieval_knn).

reference:
    xf = x.reshape(4096, 4096); xa = xf[:, ::2]; xb = xf[:, 1::2]   # [T=4096, 2048]
    sq[i,j] = |xa[:,i]|^2 + |xb[:,j]|^2 - 2 * xa[:,i].xb[:,j]
    wsum = fc_weight.sum(0); wa = wsum[::2]; wb = wsum[1::2]
    scores[i,j] = ((wa[i] + wb[j]) * sqrt(max(sq,0)))**2
                = (wa[i] + wb[j])**2 * max(sq[i,j], 0)     # sqrt cancels

Strategy (8 NeuronCores, two SPMD launches):
  Launch 1 (cross-core reductions, host combines 24 KB):
    fc_weight row-sharded (1536 rows/core, columns pre-split [even|odd])
    -> partial column sums wpart = [wa_part | wb_part]; xb column norms
    sharded over channels (256/core) -> nbsl.
  Launch 2 (main, row-sharded output): each core owns 256 rows of scores;
    (-2a)^T b in bf16 on PE, k-OUTER accumulation into all 8 PSUM banks
    so matmuls chase the chunked xb DMA stream; na computed on-device
    from (-2a)^2 * 0.25; fused fp32 DVE epilogue in-place in PSUM.
"""

import numpy as np
import ml_dtypes

import concourse.bass as bass
import concourse.tile as tile
from concourse import bacc, mybir
from concourse.bass_utils import run_bass_kernel_spmd

BF16 = mybir.dt.bfloat16
F32 = mybir.dt.float32
NP_BF16 = ml_dtypes.bfloat16
FP8 = mybir.dt.float8e4
NP_FP8 = ml_dtypes.float8_e4m3

D = 8          # cores
T = 4096       # inner (contraction) dim = B*N
KT = T // 128  # 32 k-tiles
CA = 2048      # C/2 channels
MLOC = CA // D  # 256 output rows per core
O = 12288      # fc rows
OLOC = O // D   # 1536 fc rows per core
OT = OLOC // 128  # 12 o-tiles per core
C = 4096

_cache = {}


def _new_nc():
    return bacc.Bacc("TRN2", target_bir_lowering=False, debug=False, num_devices=D)


def _build_phase1():
    """Per-core: partial fc column-sum (cols pre-split [even|odd]) +
    sharded xb column sq-norms."""
    nc = _new_nc()
    fc_d = nc.dram_tensor("fc", [128, OT, C], BF16, kind="ExternalInput").ap()
    xbs_d = nc.dram_tensor("xbs", [128, KT, MLOC], BF16, kind="ExternalInput").ap()
    wpart_d = nc.dram_tensor("wpart", [1, C], F32, kind="ExternalOutput").ap()
    nbsl_d = nc.dram_tensor("nbsl", [1, MLOC], F32, kind="ExternalOutput").ap()

    with tile.TileContext(nc) as tc:
        with (
            tc.tile_pool(name="fcp", bufs=1) as fcp,
            tc.tile_pool(name="xp", bufs=1) as xp,
            tc.tile_pool(name="small", bufs=1) as small,
            tc.tile_pool(name="stage", bufs=2) as stage,
            tc.tile_pool(name="psw", bufs=4, space="PSUM") as psw,
            tc.tile_pool(name="psn", bufs=1, space="PSUM") as psn,
        ):
            ones = small.tile([128, 1], BF16)
            nc.vector.memset(ones[:], 1.0)

            # xb slice first (small), then chunked fc load
            xbs_sb = xp.tile([128, KT, MLOC], BF16)
            nc.sync.dma_start(xbs_sb[:], xbs_d[:])
            ft = []
            for ot in range(OT):
                f = fcp.tile([128, C], BF16, tag=f"fc{ot}")
                nc.sync.dma_start(f[:], fc_d[:, ot, :])
                ft.append(f)

            # nb slice: square on ScalarE, column-sum over 32 k-tiles
            x2 = xp.tile([128, KT, MLOC], BF16)
            nc.scalar.square(x2[:], xbs_sb[:])
            psb = psn.tile([1, MLOC], F32)
            for kt in range(KT):
                nc.tensor.matmul(
                    psb[:], ones[:], x2[:, kt, :],
                    start=(kt == 0), stop=(kt == KT - 1),
                )
            st = stage.tile([1, MLOC], F32)
            nc.vector.tensor_copy(st[:], psb[:])
            nc.sync.dma_start(nbsl_d[:], st[:])

            # partial fc column sums; 4 psum banks per half
            wsb = stage.tile([1, C], F32)
            for half in range(2):
                pss = [psw.tile([1, 512], F32, name=f"psw{half}_{i}", tag="psw")
                       for i in range(4)]
                for ot in range(OT):
                    for ci, ps in enumerate(pss):
                        ch = half * 4 + ci
                        nc.tensor.matmul(
                            ps[:], ones[:],
                            ft[ot][:, ch * 512:(ch + 1) * 512],
                            start=(ot == 0), stop=(ot == OT - 1),
                        )
                for ci, ps in enumerate(pss):
                    ch = half * 4 + ci
                    nc.vector.tensor_copy(wsb[:, ch * 512:(ch + 1) * 512], ps[:])
            nc.sync.dma_start(wpart_d[:], wsb[:])

    nc.compile()
    return nc


def _build_phase2():
    """Per-core: 256 rows of scores = (wa+wb)^2 * relu(na+nb-2ab)."""
    nc = _new_nc()
    xasc_d = nc.dram_tensor("xasc", [128, KT, MLOC], FP8, kind="ExternalInput").ap()
    xbr_d = nc.dram_tensor("xbr", [128, KT, CA], FP8, kind="ExternalInput").ap()
    # wa per-partition per m-tile
    pv_d = nc.dram_tensor("pv", [128, 2], F32, kind="ExternalInput").ap()
    # packed free-axis vectors: [0, 0:CA]=wb, [0, CA:2CA]=nb
    fv_d = nc.dram_tensor("fv", [1, 2 * CA], F32, kind="ExternalInput").ap()
    out_d = nc.dram_tensor("scores", [MLOC, CA], F32, kind="ExternalOutput").ap()
    na_dram = nc.dram_tensor("na_tmp", [1, MLOC], F32).ap()

    NJ = CA // 512   # 4 column chunks
    MT = MLOC // 128  # 2 m-tiles
    KG = 2           # k-tiles per xb DMA chunk
    XG = 8           # k-tiles per xa DMA chunk

    with tile.TileContext(nc) as tc:
        with (
            tc.tile_pool(name="xap", bufs=1) as xap,
            tc.tile_pool(name="xbp", bufs=1) as xbp,
            tc.tile_pool(name="small", bufs=1) as small,
            tc.tile_pool(name="w2p", bufs=1) as w2p,
            tc.tile_pool(name="x2p", bufs=2) as x2p,
            tc.tile_pool(name="outp", bufs=2) as outp,
        ):
            # ---- input streams (emission order = DMA issue order) ----
            xac = []
            for g in range(KT // XG):
                x_t = xap.tile([128, XG, MLOC], FP8, tag=f"xa{g}")
                nc.sync.dma_start(x_t[:], xasc_d[:, g * XG:(g + 1) * XG, :])
                xac.append(x_t)
            xbt = []
            for h in range(KT // KG):
                xb_t = xbp.tile([128, KG, CA], FP8, tag=f"xb{h}")
                nc.sync.dma_start(xb_t[:], xbr_d[:, h * KG:(h + 1) * KG, :])
                xbt.append(xb_t)

            quarter = small.tile([128, 1], BF16)
            nc.vector.memset(quarter[:], 0.25)

            # ---- na from (-2a)^2 * 0.25, then DRAM roundtrip to [128,2] ----
            with tc.tile_pool(name="psna", bufs=1, space="PSUM") as psna:
                psa = psna.tile([1, MLOC], F32)
                for g in range(KT // XG):
                    x2 = x2p.tile([128, XG, MLOC], BF16, tag="x2")
                    nc.scalar.square(x2[:], xac[g][:])
                    for s in range(XG):
                        kt = g * XG + s
                        nc.tensor.matmul(
                            psa[:], quarter[:], x2[:, s, :],
                            start=(kt == 0), stop=(kt == KT - 1),
                        )
                nast = small.tile([1, MLOC], F32)
                nc.vector.tensor_copy(nast[:], psa[:])
                nc.sync.dma_start(na_dram[:], nast[:])
            nav = small.tile([128, MT], F32)
            nc.sync.dma_start(
                nav[:],
                bass.AP(tensor=na_dram.tensor, offset=0, ap=[[1, 128], [128, MT]]),
            )

            # ---- main matmul: k-OUTER accumulation, 2 x 4-bank psum tiles ----
            with tc.tile_pool(name="psmm", bufs=2, space="PSUM") as psmm:
                ps = [psmm.tile([128, NJ, 512], F32, name=f"ps{m}", tag="ps")
                      for m in range(MT)]
                for kt in range(KT):
                    h, r = divmod(kt, KG)
                    g, s = divmod(kt, XG)
                    for m in range(MT):
                        for nj in range(NJ):
                            nc.tensor.matmul(
                                ps[m][:, nj, :],
                                xac[g][:, s, m * 128:(m + 1) * 128],
                                xbt[h][:, r, nj * 512:(nj + 1) * 512],
                                start=(kt == 0), stop=(kt == KT - 1),
                            )

                # ---- epilogue vectors (issued late; DMA overlaps MM stream) ----
                pv = small.tile([128, 2], F32)
                nc.sync.dma_start(pv[:], pv_d[:])
                wb_bc = small.tile([128, CA], F32)
                nc.sync.dma_start(wb_bc[:], fv_d[0:1, 0:CA].to_broadcast([128, CA]))
                nb_bc = small.tile([128, CA], F32)
                nc.sync.dma_start(nb_bc[:], fv_d[0:1, CA:2 * CA].to_broadcast([128, CA]))
                w2 = []
                for m in range(MT):
                    w2m = w2p.tile([128, CA], F32, tag=f"w2_{m}")
                    nc.scalar.activation(
                        w2m[:], wb_bc[:],
                        mybir.ActivationFunctionType.Square,
                        bias=pv[:, m:m + 1], scale=1.0,
                    )
                    w2.append(w2m)

                # ---- epilogue: sq in-place in psum, scale, store ----
                for m in range(MT):
                    pflat = ps[m].rearrange("p a b -> p (a b)")
                    nc.vector.scalar_tensor_tensor(
                        pflat, pflat, nav[:, m:m + 1], nb_bc[:],
                        op0=mybir.AluOpType.add, op1=mybir.AluOpType.add,
                    )
                    ot = outp.tile([128, CA], F32, tag="ot")
                    nc.vector.scalar_tensor_tensor(
                        ot[:], pflat, 0.0, w2[m][:],
                        op0=mybir.AluOpType.max, op1=mybir.AluOpType.mult,
                    )
                    nc.sync.dma_start(out_d[m * 128:(m + 1) * 128, :], ot[:])

    nc.compile()
    return nc


def _p_major(a, np_dtype):
    """[n*128, cols] -> [128, n, cols] with tile index in the middle."""
    n = a.shape[0] // 128
    return np.ascontiguousarray(
        a.reshape(n, 128, a.shape[1]).transpose(1, 0, 2).astype(np_dtype)
    )


def _kernel_twolaunch(x, fc_weight, _trace=False):
    x = np.asarray(x, dtype=np.float32)
    fc = np.asarray(fc_weight, dtype=np.float32)

    xf = x.reshape(T, C)
    xa = np.ascontiguousarray(xf[:, 0::2])   # [T, CA]
    xb = np.ascontiguousarray(xf[:, 1::2])
    # deinterleave fc columns: [even | odd] so wpart = [wa_part | wb_part]
    fc_r = np.concatenate([fc[:, 0::2], fc[:, 1::2]], axis=1)

    xb_r = _p_major(xb, NP_FP8)              # [128, KT, CA]
    xa_s2 = -2.0 * xa

    # ---- launch 1 ----
    if "p1" not in _cache:
        _cache["p1"] = _build_phase1()
    nc1 = _cache["p1"]

    in_maps1 = []
    for d in range(D):
        sl = slice(d * MLOC, (d + 1) * MLOC)
        in_maps1.append({
            "fc": _p_major(fc_r[d * OLOC:(d + 1) * OLOC], NP_BF16),
            "xbs": _p_major(xb[:, sl], NP_BF16),
        })
    res1 = run_bass_kernel_spmd(nc1, in_maps1, core_ids=list(range(D)), trace=_trace)
    t1 = res1.exec_time_ns

    wsum = np.sum([res1.results[d]["wpart"][0] for d in range(D)], axis=0,
                  dtype=np.float32)                              # [C] = [wa|wb]
    nb = np.concatenate([res1.results[d]["nbsl"][0] for d in range(D)])
    wa, wb = wsum[:CA], wsum[CA:]

    # ---- launch 2 ----
    if "p2" not in _cache:
        _cache["p2"] = _build_phase2()
    nc2 = _cache["p2"]

    fv = np.concatenate([wb, nb]).reshape(1, 2 * CA).astype(np.float32)
    in_maps2 = []
    for d in range(D):
        sl = slice(d * MLOC, (d + 1) * MLOC)
        in_maps2.append({
            "xasc": _p_major(xa_s2[:, sl], NP_FP8),
            "xbr": xb_r,
            "pv": np.ascontiguousarray(wa[sl].reshape(2, 128).T).astype(np.float32),
            "fv": fv,
        })
    res2 = run_bass_kernel_spmd(nc2, in_maps2, core_ids=list(range(D)), trace=_trace)
    t2 = res2.exec_time_ns

    out = np.concatenate([res2.results[d]["scores"] for d in range(D)], axis=0)
    if _trace:
        kernel.last_times = (t1, t2)
    return out.astype(np.float32)


def _build_merged():
    """Single launch, fc column-sharded per core, odd(wb)/even(wa) halves as
    separate streams: wb half loads first so its AllGather issues early.
    Main matmul fp8; fused fp32 epilogue in PSUM."""
    nc = _new_nc()
    OTT = O // 128    # 96 fc o-tiles
    FG = 16           # o-tiles per fc DMA chunk -> 6 chunks per half
    fcb_d = nc.dram_tensor("fcb", [128, OTT, MLOC], BF16, kind="ExternalInput").ap()
    fca_d = nc.dram_tensor("fca", [128, OTT, MLOC], BF16, kind="ExternalInput").ap()
    xasc_d = nc.dram_tensor("xasc", [128, KT, MLOC], FP8, kind="ExternalInput").ap()
    xbs_d = nc.dram_tensor("xbs", [128, KT, MLOC], FP8, kind="ExternalInput").ap()
    xbr_d = nc.dram_tensor("xbr", [128, KT, CA], FP8, kind="ExternalInput").ap()
    out_d = nc.dram_tensor("scores", [MLOC, CA], F32, kind="ExternalOutput").ap()

    nb_in = nc.dram_tensor("nb_in", [1, MLOC], F32).ap()
    nb_sh = nc.dram_tensor("nb_sh", [D, MLOC], F32, addr_space="Shared").ap()
    wb_in = nc.dram_tensor("wb_in", [1, MLOC], F32).ap()
    wb_sh = nc.dram_tensor("wb_sh", [D, MLOC], F32, addr_space="Shared").ap()
    grp = [list(range(D))]

    NJ = CA // 512
    MT = MLOC // 128
    KG = 2            # k-tiles per xb DMA chunk
    XG = 8            # k-tiles per xa DMA chunk

    import contextlib
    with tile.TileContext(nc) as tc:
        es = contextlib.ExitStack()
        with es, \
             tc.tile_pool(name="xap", bufs=1) as xap, \
             tc.tile_pool(name="xsp", bufs=1) as xsp, \
             tc.tile_pool(name="xbp", bufs=1) as xbp, \
             tc.tile_pool(name="fbp", bufs=2) as fbp, \
             tc.tile_pool(name="fap", bufs=2) as fap, \
             tc.tile_pool(name="small", bufs=1) as small, \
             tc.tile_pool(name="w2p", bufs=1) as w2p, \
             tc.tile_pool(name="x2p", bufs=2) as x2p, \
             tc.tile_pool(name="outp", bufs=2) as outp, \
             tc.tile_pool(name="psm0", bufs=1, space="PSUM") as psm0:
            pse = es.enter_context(tc.tile_pool(name="pse", bufs=1, space="PSUM"))

            # ---- DMA emission: xbs, xasc, fcb (wb half), fca, xbr ----
            xbs_sb = xsp.tile([128, KT, MLOC], FP8)
            nc.sync.dma_start(xbs_sb[:], xbs_d[:])
            xac = []
            for g in range(KT // XG):
                x_t = xap.tile([128, XG, MLOC], FP8, tag=f"xa{g}")
                nc.sync.dma_start(x_t[:], xasc_d[:, g * XG:(g + 1) * XG, :])
                xac.append(x_t)
            fbt = []
            for rnd in range(OTT // FG):
                f = fbp.tile([128, FG, MLOC], BF16, tag="fcb")
                nc.sync.dma_start(f[:], fcb_d[:, rnd * FG:(rnd + 1) * FG, :])
                fbt.append(f)
            fat = []
            for rnd in range(OTT // FG):
                f = fap.tile([128, FG, MLOC], BF16, tag="fca")
                nc.sync.dma_start(f[:], fca_d[:, rnd * FG:(rnd + 1) * FG, :])
                fat.append(f)
            xbt = []
            for bi in range(KT // KG):
                xb_t = xbp.tile([128, KG, CA], FP8, tag=f"xb{bi}")
                nc.sync.dma_start(xb_t[:], xbr_d[:, bi * KG:(bi + 1) * KG, :])
                xbt.append(xb_t)

            ones = small.tile([128, 1], BF16)
            nc.vector.memset(ones[:], 1.0)
            quarter = small.tile([128, 1], BF16)
            nc.vector.memset(quarter[:], 0.25)
            onef = small.tile([1, 1], F32)
            nc.vector.memset(onef[:], 1.0)

            # ---- nb slice (feeds earliest AllGather) ----
            psb = pse.tile([1, MLOC], F32, name="psb", tag="psb")
            for g in range(KT // XG):
                x2b = x2p.tile([128, XG, MLOC], BF16, tag="x2b")
                nc.scalar.square(x2b[:], xbs_sb[:, g * XG:(g + 1) * XG, :])
                for st_ in range(XG):
                    kt = g * XG + st_
                    nc.tensor.matmul(psb[:], ones[:], x2b[:, st_, :],
                                     start=(kt == 0), stop=(kt == KT - 1))
            nbst = small.tile([1, MLOC], F32)
            nc.vector.tensor_copy(nbst[:], psb[:])
            nc.gpsimd.dma_start(nb_in[:], nbst[:])
            nc.gpsimd.collective_compute(
                "AllGather", mybir.AluOpType.bypass, replica_groups=grp,
                ins=[nb_in[:]], outs=[nb_sh[:]])

            # ---- fcb (odd cols): wb_part -> AllGather ASAP ----
            pswb = pse.tile([1, MLOC], F32, name="pswb", tag="bchain")
            for rnd in range(OTT // FG):
                for o in range(FG):
                    ot = rnd * FG + o
                    nc.tensor.matmul(pswb[:], ones[:], fbt[rnd][:, o, :],
                                     start=(ot == 0), stop=(ot == OTT - 1))
            wbst = small.tile([1, MLOC], F32)
            nc.vector.tensor_copy(wbst[:], pswb[:])
            nc.gpsimd.dma_start(wb_in[:], wbst[:])
            nc.gpsimd.collective_compute(
                "AllGather", mybir.AluOpType.bypass, replica_groups=grp,
                ins=[wb_in[:]], outs=[wb_sh[:]])
            # broadcast reads on gpsimd (gated only on the AGs)
            nb_bc = small.tile([128, CA], F32)
            nc.gpsimd.dma_start(nb_bc[:], bass.AP(tensor=nb_sh.tensor, offset=0,
                                                  ap=[[0, 128], [1, CA]]))
            wb_bc = small.tile([128, CA], F32)
            nc.gpsimd.dma_start(wb_bc[:], bass.AP(tensor=wb_sh.tensor, offset=0,
                                                  ap=[[0, 128], [1, CA]]))

            # ---- na local + transpose to [128, MT] via K=1 matmuls ----
            psa = pse.tile([1, MLOC], F32, name="psa", tag="psa")
            for g in range(KT // XG):
                x2 = x2p.tile([128, XG, MLOC], BF16, tag="x2")
                nc.scalar.square(x2[:], xac[g][:])
                for st_ in range(XG):
                    kt = g * XG + st_
                    nc.tensor.matmul(psa[:], quarter[:], x2[:, st_, :],
                                     start=(kt == 0), stop=(kt == KT - 1))
            nast = small.tile([1, MLOC], F32)
            nc.vector.tensor_copy(nast[:], psa[:])
            pst_a = pse.tile([128, MT], F32, name="pst_a", tag="wchain")
            for m in range(MT):
                nc.tensor.matmul(pst_a[:, m:m + 1],
                                 nast[0:1, m * 128:(m + 1) * 128], onef[:],
                                 start=(m == 0), stop=(m == MT - 1),
                                 skip_group_check=True)
            nav = small.tile([128, MT], F32)
            nc.vector.tensor_copy(nav[:], pst_a[:])

            # ---- fca (even cols): wa local ----
            pswa = pse.tile([1, MLOC], F32, name="pswa", tag="wchain")
            for rnd in range(OTT // FG):
                for o in range(FG):
                    ot = rnd * FG + o
                    nc.tensor.matmul(pswa[:], ones[:], fat[rnd][:, o, :],
                                     start=(ot == 0), stop=(ot == OTT - 1))
            wast = small.tile([1, MLOC], F32)
            nc.vector.tensor_copy(wast[:], pswa[:])
            pst_w = pse.tile([128, MT], F32, name="pst_w", tag="wchain")
            for m in range(MT):
                nc.tensor.matmul(pst_w[:, m:m + 1],
                                 wast[0:1, m * 128:(m + 1) * 128], onef[:],
                                 start=(m == 0), stop=(m == MT - 1),
                                 skip_group_check=True)
            wav = small.tile([128, MT], F32)
            nc.vector.tensor_copy(wav[:], pst_w[:])

            # ---- m0 matmuls (chase xbr stream) ----
            ps0 = psm0.tile([128, NJ, 512], F32, name="ps0", tag="ps")
            for kt in range(KT):
                g, s_ = divmod(kt, XG)
                h, r_ = divmod(kt, KG)
                for nj in range(NJ):
                    nc.tensor.matmul(
                        ps0[:, nj, :],
                        xac[g][:, s_, 0:128],
                        xbt[h][:, r_, nj * 512:(nj + 1) * 512],
                        start=(kt == 0), stop=(kt == KT - 1))

            es.close()

            w2 = []
            for m in range(MT):
                w2m = w2p.tile([128, CA], F32, tag=f"w2_{m}")
                nc.scalar.activation(w2m[:], wb_bc[:],
                                     mybir.ActivationFunctionType.Square,
                                     bias=wav[:, m:m + 1], scale=1.0)
                w2.append(w2m)

            with tc.tile_pool(name="psm1", bufs=1, space="PSUM") as psm1:
                ps1 = psm1.tile([128, NJ, 512], F32, name="ps1", tag="ps")
                for kt in range(KT):
                    g, s_ = divmod(kt, XG)
                    h, r_ = divmod(kt, KG)
                    for nj in range(NJ):
                        nc.tensor.matmul(
                            ps1[:, nj, :],
                            xac[g][:, s_, 128:256],
                            xbt[h][:, r_, nj * 512:(nj + 1) * 512],
                            start=(kt == 0), stop=(kt == KT - 1))

                for m, psm in ((0, ps0), (1, ps1)):
                    pflat = psm.rearrange("p a b -> p (a b)")
                    nc.vector.scalar_tensor_tensor(
                        pflat, pflat, nav[:, m:m + 1], nb_bc[:],
                        op0=mybir.AluOpType.add, op1=mybir.AluOpType.add)
                    ot = outp.tile([128, CA], F32, tag="ot")
                    nc.vector.scalar_tensor_tensor(
                        ot[:], pflat, 0.0, w2[m][:],
                        op0=mybir.AluOpType.max, op1=mybir.AluOpType.mult)
                    nc.sync.dma_start(out_d[m * 128:(m + 1) * 128, :], ot[:])

    nc.compile()
    return nc


def kernel_merged(x, fc_weight, _trace=False):
    x = np.asarray(x, dtype=np.float32)
    fc = np.asarray(fc_weight, dtype=np.float32)
    xf = x.reshape(T, C)
    xa = np.ascontiguousarray(xf[:, 0::2])
    xb = np.ascontiguousarray(xf[:, 1::2])
    xb_r = _p_major(xb, NP_FP8)
    xa_s2 = -2.0 * xa

    if "pm" not in _cache:
        _cache["pm"] = _build_merged()
    ncm = _cache["pm"]
    in_maps = []
    for d in range(D):
        sl = slice(d * MLOC, (d + 1) * MLOC)
        lo, hi = 2 * d * MLOC, 2 * (d + 1) * MLOC
        in_maps.append({
            "fcb": _p_major(np.ascontiguousarray(fc[:, lo + 1:hi:2]), NP_BF16),
            "fca": _p_major(np.ascontiguousarray(fc[:, lo:hi:2]), NP_BF16),
            "xasc": _p_major(xa_s2[:, sl], NP_FP8),
            "xbs": _p_major(xb[:, sl], NP_FP8),
            "xbr": xb_r,
        })
    res = run_bass_kernel_spmd(ncm, in_maps, core_ids=list(range(D)), trace=_trace)
    out = np.concatenate([res.results[d]["scores"] for d in range(D)], axis=0)
    if _trace:
        kernel_merged.last_times = (res.exec_time_ns,)
    return out.astype(np.float32)


def kernel(x, fc_weight):
    """Graded entrypoint: full inputs in, full [2048, 2048] scores out."""
    return kernel_merged(x, fc_weight)

